# revision 15
# baseline (speedup 1.0000x reference)
"""Trainium2 Bass kernel for the arc-projection problem.

Full-input contract: kernel(**inputs) takes the unsharded numpy inputs and
returns the full output. Internally shards the batch N=64 across 8 cores
(pure data parallel), runs one SPMD Bass kernel, and gathers.

Algorithm (matches reference._arc_projection):
  For each (sample, branch, direction) row:
    - segment vectors sv, masked lengths sl, cumsum cum, masked unit dirs w
    - project trajectory point 0 on all segments -> entry_s (one-hot argmin)
    - target_t = min(entry + traj_cum_t, total)
    - proj_c(t) = base_c + sum_{j=0..NP-1} dw_cj * relu(target_t - cum_j)
      (telescoped hinge identity: clip(x,0,sl) = relu(x) - relu(x-sl);
       dw_c0 = w_c0, dw_cj = w_cj - w_c,j-1, dw_c,NP-1 = -w_c,NP-2,
       with w masked to 0 on invalid segments so dw stays bounded)
    - the j-contraction runs on the TensorEngine: per row two matmuls
      lhsT=H[j,t] (128x128 relu matrix) x rhs=dw[j,3], accumulated in PSUM.
    - H is produced by 2 DVE tensor_scalar ops + 1 ACT relu per row, using
      PE-transposed per-row scalar columns (cum^T, (entry,total) broadcast).
    - cost = sum_t |pos_t - proj_t| via ACT sqrt + ones-matmul partition sum;
      per-sample argmin via one-hot; final gather as tiny batched DVE ops.
"""

import sys

import numpy as np

try:
    import concourse.bass as bass
except ImportError:  # pragma: no cover - container without PYTHONPATH set
    sys.path.insert(0, "/opt/trn_rl_repo")
    import concourse.bass as bass

import concourse.tile as tile
from concourse import bacc, masks, mybir
from concourse.bass_utils import run_bass_kernel_spmd

f32 = mybir.dt.float32
AT = mybir.AluOpType
AX = mybir.AxisListType
AF = mybir.ActivationFunctionType

N, T, NB, NP = 64, 128, 16, 256
NCORES = 8
NS = N // NCORES          # samples per core
NB2 = 2 * NB              # fwd + bwd branches
NSEG = NP - 1
BIG = 1.0e30
RT = 128                  # rows per partition-tile
SPT = RT // NB2           # samples per tile


def _view(t, ap_dims, extra_off=0):
    """Strided view of a tile/AP: ap_dims are [step, count] free dims after
    the partition dim (kept from t)."""
    return bass.AP(tensor=t.tensor, offset=t.offset + extra_off,
                   ap=[t.ap[0]] + ap_dims)


def _pview(t, part_dim, ap_dims, extra_off=0):
    """View with explicit partition dim replacement."""
    return bass.AP(tensor=t.tensor, offset=t.offset + extra_off,
                   ap=[part_dim] + ap_dims)


def _dview(t, ap_dims, extra_off=0):
    """Raw view of a DRAM tile: ap_dims replace all dims."""
    return bass.AP(tensor=t.tensor, offset=t.offset + extra_off, ap=ap_dims)


def build_nc(ns=NS, enable_asserts=False):
    rows = ns * NB2
    ntiles = (rows + RT - 1) // RT

    nc = bacc.Bacc("TRN2", target_bir_lowering=False, debug=False,
                   enable_asserts=enable_asserts, num_devices=NCORES)

    rp_d = nc.dram_tensor("rp", [rows, 3, NP], f32, kind="ExternalInput")
    mk_d = nc.dram_tensor("mk", [rows, NP], f32, kind="ExternalInput")
    tj_d = nc.dram_tensor("tj", [ns, 3, T], f32, kind="ExternalInput")
    out_d = nc.dram_tensor("out", [ns, T, 3], f32, kind="ExternalOutput")

    with tile.TileContext(nc) as tc:
        with (
            tc.tile_pool(name="const", bufs=1) as cp,
            tc.tile_pool(name="work", bufs=2) as wp,
            tc.tile_pool(name="row", bufs=4) as rwp,
            tc.tile_pool(name="fin", bufs=1) as fp,
            tc.tile_pool(name="ps", bufs=3, space="PSUM") as pp,
            tc.tile_pool(name="proj", bufs=2, space="PSUM") as jp,
            tc.tile_pool(name="dram", bufs=1, space="DRAM") as dp,
        ):
            ident = cp.tile([128, 128], f32)
            masks.make_identity(nc, ident[:, :])
            ones_col = cp.tile([128, 1], f32)
            nc.vector.memset(ones_col, 1.0)
            ones_row = cp.tile([1, 128], f32)
            nc.vector.memset(ones_row, 1.0)
            ones128 = cp.tile([128, 128], f32)
            nc.vector.memset(ones128, 1.0)
            # esel[s]: row-selector matrices (row 32*s all ones) so a matmul
            # lhsT=esel[s], rhs=X broadcasts X[32s, :] across all partitions
            esels = []
            for s in range(SPT):
                es_t = cp.tile([128, 128], f32, name=f"esel{s}")
                nc.gpsimd.memset(es_t, 0.0)
                nc.gpsimd.affine_select(
                    out=es_t, in_=es_t, compare_op=AT.not_equal, fill=1.0,
                    base=-NB2 * s, channel_multiplier=1, pattern=[[0, 128]])
                esels.append(es_t)

            cost_d = dp.tile([rows], f32)
            costS = fp.tile([1, rows], f32, tag="costS")

            dfs = []      # per-tile df tiles (SBUF), kept for final gather
            tposTs = []   # per-tile transposed trajectory positions

            for k in range(ntiles):
                p = min(RT, rows - k * RT)
                r0 = k * RT

                rpt = wp.tile([p, 3, NP], f32, tag="rpt")
                nc.sync.dma_start(out=rpt, in_=rp_d.ap()[r0:r0 + p])
                mt = wp.tile([p, NP], f32, tag="mt")
                nc.sync.dma_start(out=mt, in_=mk_d.ap()[r0:r0 + p])
                # trajectory of each row's sample, broadcast to its 32 rows
                tpb = wp.tile([p, 3, T], f32, tag="tpb")
                nc.sync.dma_start(out=tpb, in_=bass.AP(
                    tensor=tj_d.ap().tensor, offset=k * SPT * 3 * T,
                    ap=[[3 * T, SPT], [0, NB2], [1, 3 * T]]))

                # --- segment data ---
                sv = wp.tile([p, 3, NSEG], f32, tag="sv")
                nc.vector.tensor_sub(out=sv, in0=rpt[:, :, 1:NP],
                                     in1=rpt[:, :, 0:NSEG])
                sm = wp.tile([p, NSEG], f32, tag="sm")
                nc.gpsimd.tensor_mul(out=sm, in0=mt[:, 1:NP], in1=mt[:, 0:NSEG])
                sq3 = wp.tile([p, 3, NSEG], f32, tag="sq3")
                nc.scalar.square(out=sq3, in_=sv)
                sl2 = wp.tile([p, NSEG], f32, tag="sl2")
                nc.vector.tensor_reduce(out=sl2,
                                        in_=_view(sq3, [[1, NSEG], [NSEG, 3]]),
                                        axis=AX.X, op=AT.add)
                sl2m = wp.tile([p, NSEG], f32, tag="sl2m")
                nc.vector.tensor_mul(out=sl2m, in0=sl2, in1=sm)
                sl = wp.tile([p, NSEG], f32, tag="sl")
                nc.scalar.sqrt(out=sl, in_=sl2m)

                cum = wp.tile([p, NP], f32, tag="cum")
                zc = wp.tile([p, 1], f32, tag="zc")
                nc.vector.memset(zc, 0.0)
                nc.vector.memset(cum[:, 0:1], 0.0)
                nc.vector.tensor_tensor_scan(
                    out=cum[:, 1:NP], data0=sl, data1=_view(zc, [[0, NSEG]]),
                    initial=0.0, op0=AT.add, op1=AT.add)
                total = cum[:, NP - 1:NP]
                cumneg = wp.tile([p, NP], f32, tag="cumneg")
                nc.gpsimd.tensor_scalar(out=cumneg, in0=cum, scalar1=-1.0,
                                        scalar2=None, op0=AT.mult)
                slmax = wp.tile([p, NSEG], f32, tag="slmax")
                nc.vector.tensor_scalar(out=slmax, in0=sl, scalar1=1e-9,
                                        scalar2=None, op0=AT.max)
                rsl = wp.tile([p, NSEG], f32, tag="rsl")
                nc.vector.reciprocal(out=rsl, in_=slmax)
                rslm = wp.tile([p, NSEG], f32, tag="rslm")
                nc.gpsimd.tensor_mul(out=rslm, in0=rsl, in1=sm)
                w = wp.tile([p, 3, NSEG], f32, tag="w")
                nc.vector.tensor_mul(out=w, in0=sv,
                                     in1=_view(rslm, [[0, 3], [1, NSEG]]))
                # dw: telescoped weights over j=0..NP-1
                dw = wp.tile([p, 3, NP], f32, tag="dw")
                nc.vector.tensor_copy(out=_view(dw, [[NP, 3]]),
                                      in_=_view(w, [[NSEG, 3]]))
                nc.vector.tensor_sub(
                    out=_view(dw, [[NP, 3], [1, NSEG - 1]], extra_off=1),
                    in0=_view(w, [[NSEG, 3], [1, NSEG - 1]], extra_off=1),
                    in1=_view(w, [[NSEG, 3], [1, NSEG - 1]]))
                nc.vector.tensor_scalar(
                    out=_view(dw, [[NP, 3]], extra_off=NSEG),
                    in0=_view(w, [[NSEG, 3]], extra_off=NSEG - 1),
                    scalar1=-1.0, scalar2=None, op0=AT.mult)

                # --- project p0 on all segments; entry_s via one-hot argmin ---
                tmp3 = wp.tile([p, 3, NSEG], f32, tag="tmp3")
                for c in range(3):
                    # (a_c - p0_c) * sv_c
                    nc.vector.scalar_tensor_tensor(
                        out=tmp3[:, c, :], in0=rpt[:, c, 0:NSEG],
                        scalar=tpb[:, c, 0:1], in1=sv[:, c, :],
                        op0=AT.subtract, op1=AT.mult)
                dotn = wp.tile([p, NSEG], f32, tag="dotn")
                nc.vector.tensor_reduce(out=dotn,
                                        in_=_view(tmp3, [[1, NSEG], [NSEG, 3]]),
                                        axis=AX.X, op=AT.add)
                svd = wp.tile([p, NSEG], f32, tag="svd")
                nc.vector.tensor_scalar(out=svd, in0=sl2, scalar1=1e-12,
                                        scalar2=None, op0=AT.max)
                rsvd = wp.tile([p, NSEG], f32, tag="rsvd")
                nc.vector.reciprocal(out=rsvd, in_=svd)
                t0 = wp.tile([p, NSEG], f32, tag="t0")
                nc.vector.tensor_mul(out=t0, in0=dotn, in1=rsvd)
                # t0 = min(max(-t0, 0), 1)
                nc.vector.tensor_scalar(out=t0, in0=t0, scalar1=-1.0,
                                        scalar2=0.0, op0=AT.mult, op1=AT.max)
                nc.vector.tensor_scalar(out=t0, in0=t0, scalar1=1.0,
                                        scalar2=None, op0=AT.min)
                s3 = wp.tile([p, 3, NSEG], f32, tag="s3")
                nc.vector.tensor_mul(out=s3, in0=sv,
                                     in1=_view(t0, [[0, 3], [1, NSEG]]))
                e3 = wp.tile([p, 3, NSEG], f32, tag="e3")
                for c in range(3):
                    # (a_c - p0_c) + t0*sv_c  (= q0_c - p0_c)
                    nc.vector.scalar_tensor_tensor(
                        out=e3[:, c, :], in0=rpt[:, c, 0:NSEG],
                        scalar=tpb[:, c, 0:1], in1=s3[:, c, :],
                        op0=AT.subtract, op1=AT.add)
                e3sq = wp.tile([p, 3, NSEG], f32, tag="e3sq")
                nc.scalar.square(out=e3sq, in_=e3)
                d2 = wp.tile([p, NSEG], f32, tag="d2")
                nc.vector.tensor_reduce(out=d2,
                                        in_=_view(e3sq, [[1, NSEG], [NSEG, 3]]),
                                        axis=AX.X, op=AT.add)
                d2m = wp.tile([p, NSEG], f32, tag="d2m")
                # d2m = d2 + (1-sm)*BIG  (sm is exactly 0/1)
                nc.vector.tensor_scalar(out=d2m, in0=sm, scalar1=1.0,
                                        scalar2=-BIG, op0=AT.subtract,
                                        op1=AT.mult)
                nc.vector.tensor_add(out=d2m, in0=d2m, in1=d2)
                dmin = wp.tile([p, 1], f32, tag="dmin")
                nc.vector.tensor_reduce(out=dmin, in_=d2m, axis=AX.X, op=AT.min)
                ohseg = wp.tile([p, NSEG], f32, tag="ohseg")
                nc.vector.tensor_scalar(out=ohseg, in0=d2m, scalar1=dmin,
                                        scalar2=None, op0=AT.is_equal)
                # keep only the FIRST hot (ties are structural), as jnp.argmin
                pmax = wp.tile([p, NSEG], f32, tag="pmax")
                nc.vector.tensor_tensor_scan(
                    out=pmax, data0=ohseg, data1=_view(zc, [[0, NSEG]]),
                    initial=0.0, op0=AT.max, op1=AT.add)
                nc.vector.tensor_copy(out=ohseg[:, 0:1], in_=pmax[:, 0:1])
                nc.vector.tensor_sub(out=ohseg[:, 1:NSEG], in0=pmax[:, 1:NSEG],
                                     in1=pmax[:, 0:NSEG - 1])
                es = wp.tile([p, NSEG], f32, tag="es")
                nc.vector.tensor_mul(out=es, in0=t0, in1=sl)
                nc.vector.tensor_add(out=es, in0=es, in1=cum[:, 0:NSEG])
                entry = wp.tile([p, 1], f32, tag="entry")
                junk0 = wp.tile([p, NSEG], f32, tag="junk0")
                nc.vector.scalar_tensor_tensor(
                    out=junk0, in0=ohseg, scalar=1.0, in1=es,
                    op0=AT.mult, op1=AT.mult, accum_out=entry)

                # --- base point rp[first valid segment] ---
                ohf = wp.tile([p, NSEG], f32, tag="ohf")
                nc.vector.tensor_copy(out=ohf[:, 0:1], in_=sm[:, 0:1])
                nc.vector.tensor_sub(out=ohf[:, 1:NSEG], in0=sm[:, 1:NSEG],
                                     in1=sm[:, 0:NSEG - 1])
                nc.vector.tensor_scalar(out=ohf, in0=ohf, scalar1=0.0,
                                        scalar2=None, op0=AT.max)
                base3 = wp.tile([p, 3], f32, tag="base3")
                for c in range(3):
                    nc.vector.scalar_tensor_tensor(
                        out=junk0, in0=ohf, scalar=1.0, in1=rpt[:, c, 0:NSEG],
                        op0=AT.mult, op1=AT.mult,
                        accum_out=base3[:, c:c + 1])

                # --- trajectory cumulative arc length ---
                td = wp.tile([p, 3, T - 1], f32, tag="td")
                nc.vector.tensor_sub(out=td, in0=tpb[:, :, 1:T],
                                     in1=tpb[:, :, 0:T - 1])
                td2 = wp.tile([p, 3, T - 1], f32, tag="td2")
                nc.scalar.square(out=td2, in_=td)
                tl2 = wp.tile([p, T - 1], f32, tag="tl2")
                nc.vector.tensor_reduce(out=tl2,
                                        in_=_view(td2, [[1, T - 1], [T - 1, 3]]),
                                        axis=AX.X, op=AT.add)
                tl = wp.tile([p, T - 1], f32, tag="tl")
                nc.scalar.sqrt(out=tl, in_=tl2)
                tcum = wp.tile([p, T], f32, tag="tcum")
                nc.vector.memset(tcum[:, 0:1], 0.0)
                nc.vector.tensor_tensor_scan(
                    out=tcum[:, 1:T], data0=tl, data1=_view(zc, [[0, T - 1]]),
                    initial=0.0, op0=AT.add, op1=AT.add)

                # --- posb = pos - base (per channel) ---
                posb = wp.tile([p, 3, T], f32, tag="posb")
                for c in range(3):
                    nc.vector.tensor_scalar(
                        out=posb[:, c, :], in0=tpb[:, c, :],
                        scalar1=base3[:, c:c + 1], scalar2=None,
                        op0=AT.subtract)

                # --- PE transposes to j-major / t-major layouts ---
                def _copy(eng, dst, src):
                    if eng is nc.scalar:
                        nc.scalar.copy(out=dst, in_=src)
                    else:
                        eng.tensor_copy(out=dst, in_=src)

                def transpose_to(dst, src_ap, eng=nc.vector):
                    ps = pp.tile([128, 128], f32, tag="tmp")
                    pslice = ps[:src_ap.ap[1][1], :p] if len(src_ap.ap) > 1 \
                        else ps
                    nc.tensor.transpose(out=pslice, in_=src_ap,
                                        identity=ident[:p, :p])
                    _copy(eng, dst, pslice)

                cumT0 = wp.tile([128, p], f32, tag="cumT0")
                transpose_to(cumT0, cum[:, 0:128])
                ncumT1 = wp.tile([128, p], f32, tag="ncumT1")
                transpose_to(ncumT1, cumneg[:, 128:256])
                dwT0 = wp.tile([128, 3, p], f32, tag="dwT0")
                dwT1 = wp.tile([128, 3, p], f32, tag="dwT1")
                for c in range(3):
                    transpose_to(dwT0[:, c, :], dw[:, c, 0:128],
                                 eng=(nc.vector, nc.scalar, nc.vector)[c])
                    transpose_to(dwT1[:, c, :], dw[:, c, 128:256],
                                 eng=(nc.scalar, nc.vector, nc.scalar)[c])
                posbT = wp.tile([T, 3, p], f32, tag="posbT")
                tposT = wp.tile([T, 3, p], f32, tag="tposT")
                for c in range(3):
                    transpose_to(posbT[:, c, :], posb[:, c, :])
                    transpose_to(tposT[:, c, :], tpb[:, c, :],
                                 eng=nc.scalar)
                tposTs.append(tposT)

                # --- broadcasts across partitions ---
                # per-row scalars: diag(v) built on DVE, then ones.T @ diag(v)
                # puts v_r into column r on every partition.
                def colbcast(dst, col, tag, eng=nc.vector):
                    dg = wp.tile([p, 128], f32, tag=tag)
                    nc.vector.tensor_scalar(out=dg, in0=ident[:p, :],
                                            scalar1=col, scalar2=None,
                                            op0=AT.mult)
                    ps = pp.tile([128, p], f32, tag="tmp")
                    nc.tensor.matmul(out=ps, lhsT=ones128[:p, :], rhs=dg,
                                     start=True, stop=True)
                    _copy(eng, dst, ps)

                entry_b = wp.tile([128, p], f32, tag="entry_b")
                colbcast(entry_b, entry, "dg_e")
                total_b = wp.tile([128, p], f32, tag="total_b")
                colbcast(total_b, total, "dg_t", eng=nc.scalar)
                # per-sample trajectory cumsum broadcast: esel[s] @ tcum
                tcum_bs = []
                for s in range(SPT):
                    tb = wp.tile([128, T], f32, tag=f"tcum_b{s}")
                    ps = pp.tile([128, T], f32, tag="tmp")
                    nc.tensor.matmul(out=ps, lhsT=esels[s][:p, :], rhs=tcum,
                                     start=True, stop=True)
                    _copy(nc.scalar if s % 2 else nc.vector, tb, ps)
                    tcum_bs.append(tb)

                # --- per-row: H matrices + TensorE contraction ---
                projbank = jp.tile([T, 3 * p], f32, tag="projbank")
                for r in range(p):
                    s = r // NB2
                    tgt = rwp.tile([128, T], f32, tag="tgt")
                    nc.vector.tensor_scalar(
                        out=tgt, in0=tcum_bs[s],
                        scalar1=entry_b[:, r:r + 1],
                        scalar2=total_b[:, r:r + 1],
                        op0=AT.add, op1=AT.min)
                    h0 = rwp.tile([128, T], f32, tag="h0")
                    nc.vector.tensor_scalar(
                        out=h0, in0=tgt, scalar1=cumT0[:, r:r + 1],
                        scalar2=0.0, op0=AT.subtract, op1=AT.max)
                    h1 = rwp.tile([128, T], f32, tag="h1")
                    nc.scalar.activation(
                        out=h1, in_=tgt, func=AF.Relu,
                        bias=ncumT1[:, r:r + 1], scale=1.0)
                    pslice = projbank[:, 3 * r:3 * r + 3]
                    nc.tensor.matmul(out=pslice, lhsT=h0,
                                     rhs=_view(dwT0, [[p, 3]], extra_off=r),
                                     start=True, stop=False)
                    nc.tensor.matmul(out=pslice, lhsT=h1,
                                     rhs=_view(dwT1, [[p, 3]], extra_off=r),
                                     start=False, stop=True)

                # --- epilogue: df = proj - posb (t-major), cost via matmul ---
                df = wp.tile([T, 3 * p], f32, tag="df")
                nc.vector.tensor_tensor(
                    out=_view(df, [[3, p], [1, 3]]),
                    in0=_view(projbank, [[3, p], [1, 3]]),
                    in1=_view(posbT, [[1, p], [p, 3]]),
                    op=AT.subtract)
                dfs.append(df)
                df2 = wp.tile([T, 3 * p], f32, tag="df2")
                nc.gpsimd.tensor_mul(out=df2, in0=df, in1=df)
                dd = wp.tile([T, p], f32, tag="dd")
                nc.vector.tensor_reduce(out=dd,
                                        in_=_view(df2, [[3, p], [1, 3]]),
                                        axis=AX.X, op=AT.add)
                dist = wp.tile([T, p], f32, tag="dist")
                nc.scalar.sqrt(out=dist, in_=dd)
                cps = pp.tile([1, p], f32, tag="tmp")
                nc.tensor.matmul(out=cps, lhsT=ones_col, rhs=dist,
                                 start=True, stop=True)
                nc.vector.tensor_copy(out=costS[:, r0:r0 + p], in_=cps)

            # --- per-sample argmin over branches ---
            nc.sync.dma_start(out=_dview(cost_d, [[1, rows]]), in_=costS)
            cost8 = fp.tile([ns, NB2], f32, tag="cost8")
            nc.sync.dma_start(out=cost8,
                              in_=_dview(cost_d, [[NB2, ns], [1, NB2]]))
            cmin = fp.tile([ns, 1], f32, tag="cmin")
            nc.vector.tensor_reduce(out=cmin, in_=cost8, axis=AX.X, op=AT.min)
            oh8 = fp.tile([ns, NB2], f32, tag="oh8")
            nc.vector.tensor_scalar(out=oh8, in0=cost8, scalar1=cmin,
                                    scalar2=None, op0=AT.is_equal)
            zc8 = fp.tile([ns, 1], f32, tag="zc8")
            nc.vector.memset(zc8, 0.0)
            pm8 = fp.tile([ns, NB2], f32, tag="pm8")
            nc.vector.tensor_tensor_scan(
                out=pm8, data0=oh8, data1=_view(zc8, [[0, NB2]]),
                initial=0.0, op0=AT.max, op1=AT.add)
            nc.vector.tensor_copy(out=oh8[:, 0:1], in_=pm8[:, 0:1])
            nc.vector.tensor_sub(out=oh8[:, 1:NB2], in0=pm8[:, 1:NB2],
                                 in1=pm8[:, 0:NB2 - 1])
            # roundtrip to a single-partition layout (matmul rhs needs base 0)
            oh_d = dp.tile([rows], f32)
            nc.sync.dma_start(out=_dview(oh_d, [[NB2, ns], [1, NB2]]),
                              in_=oh8)
            ohS = fp.tile([1, rows], f32, tag="ohS")
            nc.sync.dma_start(out=ohS, in_=_dview(oh_d, [[1, rows]]))

            # --- gather best branch per sample: out = df[best] + pos ---
            for n in range(ns):
                k = n // SPT
                rl = (n % SPT) * NB2   # local row offset within tile
                ohb = pp.tile([128, NB2], f32, tag="tmp")
                nc.tensor.matmul(out=ohb, lhsT=ones_row,
                                 rhs=ohS[:, NB2 * n:NB2 * (n + 1)],
                                 start=True, stop=True)
                pm = fp.tile([T, 3, NB2], f32, tag=f"pm{n % 2}")
                nc.vector.tensor_tensor(
                    out=pm,
                    in0=_view(dfs[k], [[1, 3], [3, NB2]], extra_off=3 * rl),
                    in1=_view(ohb, [[0, 3], [1, NB2]]),
                    op=AT.mult)
                outn = fp.tile([T, 3], f32, tag=f"outn{n % 2}")
                nc.vector.tensor_reduce(out=outn,
                                        in_=_view(pm, [[NB2, 3], [1, NB2]]),
                                        axis=AX.X, op=AT.add)
                outf = fp.tile([T, 3], f32, tag=f"outf{n % 2}")
                nc.vector.tensor_tensor(
                    out=outf, in0=outn,
                    in1=_view(tposTs[k], [[RT, 3]], extra_off=rl),
                    op=AT.add)
                nc.sync.dma_start(out=out_d.ap()[n], in_=outf)

    nc.compile()
    return nc


def marshal_inputs(selected_traj, road_points, road_mask):
    """Host-side layout marshaling (permutations/casts only): per-core input
    dicts with fwd+bwd branch rows and planar (xyz-major) layouts."""
    st = np.ascontiguousarray(selected_traj, dtype=np.float32)
    rp = np.ascontiguousarray(road_points, dtype=np.float32)
    rm = np.asarray(road_mask)

    rp_ext = np.concatenate([rp, rp[:, :, ::-1, :]], axis=1)        # [N,NB2,NP,3]
    rp_ext = np.ascontiguousarray(rp_ext.transpose(0, 1, 3, 2))     # [N,NB2,3,NP]
    mk_ext = np.concatenate([rm, rm[:, :, ::-1]], axis=1).astype(np.float32)
    tj = np.ascontiguousarray(st.transpose(0, 2, 1))                # [N,3,T]

    in_maps = []
    for c in range(NCORES):
        s = slice(c * NS, (c + 1) * NS)
        in_maps.append({
            "rp": np.ascontiguousarray(rp_ext[s]).reshape(NS * NB2, 3, NP),
            "mk": np.ascontiguousarray(mk_ext[s]).reshape(NS * NB2, NP),
            "tj": np.ascontiguousarray(tj[s]),
        })
    return in_maps


_NC = None


def kernel(selected_traj, road_points, road_mask):
    global _NC
    if _NC is None:
        _NC = build_nc()
    in_maps = marshal_inputs(selected_traj, road_points, road_mask)
    res = run_bass_kernel_spmd(_NC, in_maps, core_ids=list(range(NCORES)))
    out = np.concatenate([r["out"] for r in res.results], axis=0)
    return out.astype(np.float32)


# revision 16
# speedup vs baseline: 1.1648x; 1.1648x over previous
"""Trainium2 Bass kernel for the arc-projection problem.

Full-input contract: kernel(**inputs) takes the unsharded numpy inputs and
returns the full output. Internally shards the batch N=64 across 8 cores
(pure data parallel), runs one SPMD Bass kernel, and gathers.

Algorithm (matches reference._arc_projection):
  For each (sample, branch, direction) row:
    - segment vectors sv, masked lengths sl, cumsum cum, masked unit dirs w
    - project trajectory point 0 on all segments -> entry_s (one-hot argmin)
    - target_t = min(entry + traj_cum_t, total)
    - proj_c(t) = base_c + sum_{j=0..NP-1} dw_cj * relu(target_t - cum_j)
      (telescoped hinge identity: clip(x,0,sl) = relu(x) - relu(x-sl);
       dw_c0 = w_c0, dw_cj = w_cj - w_c,j-1, dw_c,NP-1 = -w_c,NP-2,
       with w masked to 0 on invalid segments so dw stays bounded)
    - the j-contraction runs on the TensorEngine: per row two matmuls
      lhsT=H[j,t] (128x128 relu matrix) x rhs=dw[j,3], accumulated in PSUM.
    - H is produced by 2 DVE tensor_scalar ops + 1 ACT relu per row, using
      PE-transposed per-row scalar columns (cum^T, (entry,total) broadcast).
    - cost = sum_t |pos_t - proj_t| via ACT sqrt + ones-matmul partition sum;
      per-sample argmin via one-hot; final gather as tiny batched DVE ops.
"""

import sys

import numpy as np

try:
    import concourse.bass as bass
except ImportError:  # pragma: no cover - container without PYTHONPATH set
    sys.path.insert(0, "/opt/trn_rl_repo")
    import concourse.bass as bass

import concourse.tile as tile
from concourse import bacc, masks, mybir
from concourse.bass_utils import run_bass_kernel_spmd

f32 = mybir.dt.float32
AT = mybir.AluOpType
AX = mybir.AxisListType
AF = mybir.ActivationFunctionType

N, T, NB, NP = 64, 128, 16, 256
NCORES = 8
NS = N // NCORES          # samples per core
NB2 = 2 * NB              # fwd + bwd branches
NSEG = NP - 1
BIG = 1.0e30
RT = 128                  # rows per partition-tile
SPT = RT // NB2           # samples per tile


def _view(t, ap_dims, extra_off=0):
    """Strided view of a tile/AP: ap_dims are [step, count] free dims after
    the partition dim (kept from t)."""
    return bass.AP(tensor=t.tensor, offset=t.offset + extra_off,
                   ap=[t.ap[0]] + ap_dims)


def _pview(t, part_dim, ap_dims, extra_off=0):
    """View with explicit partition dim replacement."""
    return bass.AP(tensor=t.tensor, offset=t.offset + extra_off,
                   ap=[part_dim] + ap_dims)


def _dview(t, ap_dims, extra_off=0):
    """Raw view of a DRAM tile: ap_dims replace all dims."""
    return bass.AP(tensor=t.tensor, offset=t.offset + extra_off, ap=ap_dims)


def build_nc(ns=NS, enable_asserts=False):
    rows = ns * NB2
    ntiles = (rows + RT - 1) // RT

    nc = bacc.Bacc("TRN2", target_bir_lowering=False, debug=False,
                   enable_asserts=enable_asserts, num_devices=NCORES)

    rp_d = nc.dram_tensor("rp", [rows, 3, NP], f32, kind="ExternalInput")
    mk_d = nc.dram_tensor("mk", [rows, NP], f32, kind="ExternalInput")
    tj_d = nc.dram_tensor("tj", [ns, 3, T], f32, kind="ExternalInput")
    out_d = nc.dram_tensor("out", [ns, T, 3], f32, kind="ExternalOutput")

    with tile.TileContext(nc) as tc:
        with (
            tc.tile_pool(name="const", bufs=1) as cp,
            tc.tile_pool(name="work", bufs=2) as wp,
            tc.tile_pool(name="row", bufs=8) as rwp,
            tc.tile_pool(name="fin", bufs=1) as fp,
            tc.tile_pool(name="ps", bufs=4, space="PSUM") as pp,
            tc.tile_pool(name="proj", bufs=2, space="PSUM") as jp,
            tc.tile_pool(name="dram", bufs=1, space="DRAM") as dp,
        ):
            ident = cp.tile([128, 128], f32)
            masks.make_identity(nc, ident[:, :])
            ones_col = cp.tile([128, 1], f32)
            nc.vector.memset(ones_col, 1.0)
            ones_row = cp.tile([1, 128], f32)
            nc.vector.memset(ones_row, 1.0)
            ones128 = cp.tile([128, 128], f32)
            nc.vector.memset(ones128, 1.0)
            # esel[s]: row-selector matrices (row 32*s all ones) so a matmul
            # lhsT=esel[s], rhs=X broadcasts X[32s, :] across all partitions
            esels = []
            for s in range(SPT):
                es_t = cp.tile([128, 128], f32, name=f"esel{s}")
                nc.gpsimd.memset(es_t, 0.0)
                nc.gpsimd.affine_select(
                    out=es_t, in_=es_t, compare_op=AT.not_equal, fill=1.0,
                    base=-NB2 * s, channel_multiplier=1, pattern=[[0, 128]])
                esels.append(es_t)

            cost_d = dp.tile([rows], f32)
            costS = fp.tile([1, rows], f32, tag="costS")

            dfs = []      # per-tile df tiles (SBUF), kept for final gather
            tposTs = []   # per-tile transposed trajectory positions

            for k in range(ntiles):
                p = min(RT, rows - k * RT)
                r0 = k * RT

                rpt = wp.tile([p, 3, NP], f32, tag="rpt")
                nc.sync.dma_start(out=rpt, in_=rp_d.ap()[r0:r0 + p])
                mt = wp.tile([p, NP], f32, tag="mt")
                nc.sync.dma_start(out=mt, in_=mk_d.ap()[r0:r0 + p])
                # trajectory of each row's sample, broadcast to its 32 rows
                tpb = wp.tile([p, 3, T], f32, tag="tpb")
                nc.sync.dma_start(out=tpb, in_=bass.AP(
                    tensor=tj_d.ap().tensor, offset=k * SPT * 3 * T,
                    ap=[[3 * T, SPT], [0, NB2], [1, 3 * T]]))

                # --- segment data ---
                sv = wp.tile([p, 3, NSEG], f32, tag="sv")
                nc.vector.tensor_sub(out=sv, in0=rpt[:, :, 1:NP],
                                     in1=rpt[:, :, 0:NSEG])
                sm = wp.tile([p, NSEG], f32, tag="sm")
                nc.gpsimd.tensor_mul(out=sm, in0=mt[:, 1:NP], in1=mt[:, 0:NSEG])
                sq3 = wp.tile([p, 3, NSEG], f32, tag="sq3")
                nc.scalar.square(out=sq3, in_=sv)
                sl2 = wp.tile([p, NSEG], f32, tag="sl2")
                nc.vector.tensor_reduce(out=sl2,
                                        in_=_view(sq3, [[1, NSEG], [NSEG, 3]]),
                                        axis=AX.X, op=AT.add)
                sl2m = wp.tile([p, NSEG], f32, tag="sl2m")
                nc.vector.tensor_mul(out=sl2m, in0=sl2, in1=sm)
                sl = wp.tile([p, NSEG], f32, tag="sl")
                nc.scalar.sqrt(out=sl, in_=sl2m)

                cum = wp.tile([p, NP], f32, tag="cum")
                zc = wp.tile([p, 1], f32, tag="zc")
                nc.vector.memset(zc, 0.0)
                nc.vector.memset(cum[:, 0:1], 0.0)
                nc.vector.tensor_tensor_scan(
                    out=cum[:, 1:NP], data0=sl, data1=_view(zc, [[0, NSEG]]),
                    initial=0.0, op0=AT.add, op1=AT.add)
                total = cum[:, NP - 1:NP]
                cumneg = wp.tile([p, NP], f32, tag="cumneg")
                nc.gpsimd.tensor_scalar(out=cumneg, in0=cum, scalar1=-1.0,
                                        scalar2=None, op0=AT.mult)
                slmax = wp.tile([p, NSEG], f32, tag="slmax")
                nc.vector.tensor_scalar(out=slmax, in0=sl, scalar1=1e-9,
                                        scalar2=None, op0=AT.max)
                rsl = wp.tile([p, NSEG], f32, tag="rsl")
                nc.vector.reciprocal(out=rsl, in_=slmax)
                rslm = wp.tile([p, NSEG], f32, tag="rslm")
                nc.gpsimd.tensor_mul(out=rslm, in0=rsl, in1=sm)
                w = wp.tile([p, 3, NSEG], f32, tag="w")
                nc.vector.tensor_mul(out=w, in0=sv,
                                     in1=_view(rslm, [[0, 3], [1, NSEG]]))
                # dw: telescoped weights over j=0..NP-1
                dw = wp.tile([p, 3, NP], f32, tag="dw")
                nc.vector.tensor_copy(out=_view(dw, [[NP, 3]]),
                                      in_=_view(w, [[NSEG, 3]]))
                nc.vector.tensor_sub(
                    out=_view(dw, [[NP, 3], [1, NSEG - 1]], extra_off=1),
                    in0=_view(w, [[NSEG, 3], [1, NSEG - 1]], extra_off=1),
                    in1=_view(w, [[NSEG, 3], [1, NSEG - 1]]))
                nc.vector.tensor_scalar(
                    out=_view(dw, [[NP, 3]], extra_off=NSEG),
                    in0=_view(w, [[NSEG, 3]], extra_off=NSEG - 1),
                    scalar1=-1.0, scalar2=None, op0=AT.mult)

                # --- project p0 on all segments; entry_s via one-hot argmin ---
                tmp3 = wp.tile([p, 3, NSEG], f32, tag="tmp3")
                for c in range(3):
                    # (a_c - p0_c) * sv_c
                    nc.vector.scalar_tensor_tensor(
                        out=tmp3[:, c, :], in0=rpt[:, c, 0:NSEG],
                        scalar=tpb[:, c, 0:1], in1=sv[:, c, :],
                        op0=AT.subtract, op1=AT.mult)
                dotn = wp.tile([p, NSEG], f32, tag="dotn")
                nc.vector.tensor_reduce(out=dotn,
                                        in_=_view(tmp3, [[1, NSEG], [NSEG, 3]]),
                                        axis=AX.X, op=AT.add)
                svd = wp.tile([p, NSEG], f32, tag="svd")
                nc.vector.tensor_scalar(out=svd, in0=sl2, scalar1=1e-12,
                                        scalar2=None, op0=AT.max)
                rsvd = wp.tile([p, NSEG], f32, tag="rsvd")
                nc.vector.reciprocal(out=rsvd, in_=svd)
                t0 = wp.tile([p, NSEG], f32, tag="t0")
                nc.vector.tensor_mul(out=t0, in0=dotn, in1=rsvd)
                # t0 = min(max(-t0, 0), 1)
                nc.vector.tensor_scalar(out=t0, in0=t0, scalar1=-1.0,
                                        scalar2=0.0, op0=AT.mult, op1=AT.max)
                nc.vector.tensor_scalar(out=t0, in0=t0, scalar1=1.0,
                                        scalar2=None, op0=AT.min)
                s3 = wp.tile([p, 3, NSEG], f32, tag="s3")
                nc.vector.tensor_mul(out=s3, in0=sv,
                                     in1=_view(t0, [[0, 3], [1, NSEG]]))
                e3 = wp.tile([p, 3, NSEG], f32, tag="e3")
                for c in range(3):
                    # (a_c - p0_c) + t0*sv_c  (= q0_c - p0_c)
                    nc.vector.scalar_tensor_tensor(
                        out=e3[:, c, :], in0=rpt[:, c, 0:NSEG],
                        scalar=tpb[:, c, 0:1], in1=s3[:, c, :],
                        op0=AT.subtract, op1=AT.add)
                e3sq = wp.tile([p, 3, NSEG], f32, tag="e3sq")
                nc.scalar.square(out=e3sq, in_=e3)
                d2 = wp.tile([p, NSEG], f32, tag="d2")
                nc.vector.tensor_reduce(out=d2,
                                        in_=_view(e3sq, [[1, NSEG], [NSEG, 3]]),
                                        axis=AX.X, op=AT.add)
                d2m = wp.tile([p, NSEG], f32, tag="d2m")
                # d2m = d2 + (1-sm)*BIG  (sm is exactly 0/1)
                nc.vector.tensor_scalar(out=d2m, in0=sm, scalar1=1.0,
                                        scalar2=-BIG, op0=AT.subtract,
                                        op1=AT.mult)
                nc.vector.tensor_add(out=d2m, in0=d2m, in1=d2)
                dmin = wp.tile([p, 1], f32, tag="dmin")
                nc.vector.tensor_reduce(out=dmin, in_=d2m, axis=AX.X, op=AT.min)
                ohseg = wp.tile([p, NSEG], f32, tag="ohseg")
                nc.vector.tensor_scalar(out=ohseg, in0=d2m, scalar1=dmin,
                                        scalar2=None, op0=AT.is_equal)
                # keep only the FIRST hot (ties are structural), as jnp.argmin
                pmax = wp.tile([p, NSEG], f32, tag="pmax")
                nc.vector.tensor_tensor_scan(
                    out=pmax, data0=ohseg, data1=_view(zc, [[0, NSEG]]),
                    initial=0.0, op0=AT.max, op1=AT.add)
                nc.vector.tensor_copy(out=ohseg[:, 0:1], in_=pmax[:, 0:1])
                nc.vector.tensor_sub(out=ohseg[:, 1:NSEG], in0=pmax[:, 1:NSEG],
                                     in1=pmax[:, 0:NSEG - 1])
                es = wp.tile([p, NSEG], f32, tag="es")
                nc.vector.tensor_mul(out=es, in0=t0, in1=sl)
                nc.vector.tensor_add(out=es, in0=es, in1=cum[:, 0:NSEG])
                entry = wp.tile([p, 1], f32, tag="entry")
                junk0 = wp.tile([p, NSEG], f32, tag="junk0")
                nc.vector.scalar_tensor_tensor(
                    out=junk0, in0=ohseg, scalar=1.0, in1=es,
                    op0=AT.mult, op1=AT.mult, accum_out=entry)

                # --- base point rp[first valid segment] ---
                ohf = wp.tile([p, NSEG], f32, tag="ohf")
                nc.vector.tensor_copy(out=ohf[:, 0:1], in_=sm[:, 0:1])
                nc.vector.tensor_sub(out=ohf[:, 1:NSEG], in0=sm[:, 1:NSEG],
                                     in1=sm[:, 0:NSEG - 1])
                nc.vector.tensor_scalar(out=ohf, in0=ohf, scalar1=0.0,
                                        scalar2=None, op0=AT.max)
                base3 = wp.tile([p, 3], f32, tag="base3")
                for c in range(3):
                    nc.vector.scalar_tensor_tensor(
                        out=junk0, in0=ohf, scalar=1.0, in1=rpt[:, c, 0:NSEG],
                        op0=AT.mult, op1=AT.mult,
                        accum_out=base3[:, c:c + 1])

                # --- trajectory cumulative arc length ---
                td = wp.tile([p, 3, T - 1], f32, tag="td")
                nc.vector.tensor_sub(out=td, in0=tpb[:, :, 1:T],
                                     in1=tpb[:, :, 0:T - 1])
                td2 = wp.tile([p, 3, T - 1], f32, tag="td2")
                nc.scalar.square(out=td2, in_=td)
                tl2 = wp.tile([p, T - 1], f32, tag="tl2")
                nc.vector.tensor_reduce(out=tl2,
                                        in_=_view(td2, [[1, T - 1], [T - 1, 3]]),
                                        axis=AX.X, op=AT.add)
                tl = wp.tile([p, T - 1], f32, tag="tl")
                nc.scalar.sqrt(out=tl, in_=tl2)
                tcum = wp.tile([p, T], f32, tag="tcum")
                nc.vector.memset(tcum[:, 0:1], 0.0)
                nc.vector.tensor_tensor_scan(
                    out=tcum[:, 1:T], data0=tl, data1=_view(zc, [[0, T - 1]]),
                    initial=0.0, op0=AT.add, op1=AT.add)

                # --- posb = pos - base (per channel) ---
                posb = wp.tile([p, 3, T], f32, tag="posb")
                for c in range(3):
                    nc.vector.tensor_scalar(
                        out=posb[:, c, :], in0=tpb[:, c, :],
                        scalar1=base3[:, c:c + 1], scalar2=None,
                        op0=AT.subtract)

                # --- PE transposes to j-major / t-major layouts ---
                def _copy(eng, dst, src):
                    if eng is nc.scalar:
                        nc.scalar.copy(out=dst, in_=src)
                    else:
                        eng.tensor_copy(out=dst, in_=src)

                def transpose_to(dst, src_ap, eng=nc.vector):
                    ps = pp.tile([128, 128], f32, tag="tmp")
                    pslice = ps[:src_ap.ap[1][1], :p] if len(src_ap.ap) > 1 \
                        else ps
                    nc.tensor.transpose(out=pslice, in_=src_ap,
                                        identity=ident[:p, :p])
                    _copy(eng, dst, pslice)

                cumT0 = wp.tile([128, p], f32, tag="cumT0")
                transpose_to(cumT0, cum[:, 0:128])
                ncumT1 = wp.tile([128, p], f32, tag="ncumT1")
                transpose_to(ncumT1, cumneg[:, 128:256])
                dwT0 = wp.tile([128, 3, p], f32, tag="dwT0")
                dwT1 = wp.tile([128, 3, p], f32, tag="dwT1")
                for c in range(3):
                    transpose_to(dwT0[:, c, :], dw[:, c, 0:128],
                                 eng=(nc.vector, nc.scalar, nc.vector)[c])
                    transpose_to(dwT1[:, c, :], dw[:, c, 128:256],
                                 eng=(nc.scalar, nc.vector, nc.scalar)[c])
                posbT = wp.tile([T, 3, p], f32, tag="posbT")
                tposT = wp.tile([T, 3, p], f32, tag="tposT")
                for c in range(3):
                    transpose_to(posbT[:, c, :], posb[:, c, :])
                    transpose_to(tposT[:, c, :], tpb[:, c, :],
                                 eng=nc.scalar)
                tposTs.append(tposT)

                # --- broadcasts across partitions ---
                # per-row scalars: diag(v) built on DVE, then ones.T @ diag(v)
                # puts v_r into column r on every partition.
                def colbcast(dst, col, tag, eng=nc.vector):
                    dg = wp.tile([p, 128], f32, tag=tag)
                    nc.vector.tensor_scalar(out=dg, in0=ident[:p, :],
                                            scalar1=col, scalar2=None,
                                            op0=AT.mult)
                    ps = pp.tile([128, p], f32, tag="tmp")
                    nc.tensor.matmul(out=ps, lhsT=ones128[:p, :], rhs=dg,
                                     start=True, stop=True)
                    _copy(eng, dst, ps)

                entry_b = wp.tile([128, p], f32, tag="entry_b")
                colbcast(entry_b, entry, "dg_e")
                total_b = wp.tile([128, p], f32, tag="total_b")
                colbcast(total_b, total, "dg_t", eng=nc.scalar)
                # per-sample trajectory cumsum broadcast: esel[s] @ tcum
                tcum_bs = []
                for s in range(SPT):
                    tb = wp.tile([128, T], f32, tag=f"tcum_b{s}")
                    ps = pp.tile([128, T], f32, tag="tmp")
                    nc.tensor.matmul(out=ps, lhsT=esels[s][:p, :], rhs=tcum,
                                     start=True, stop=True)
                    _copy(nc.scalar if s % 2 else nc.vector, tb, ps)
                    tcum_bs.append(tb)

                # --- per-row: H matrices + TensorE contraction ---
                projbank = jp.tile([T, 3 * p], f32, tag="projbank")
                for r in range(p):
                    s = r // NB2
                    tgt = rwp.tile([128, T], f32, tag="tgt")
                    nc.vector.tensor_scalar(
                        out=tgt, in0=tcum_bs[s],
                        scalar1=entry_b[:, r:r + 1],
                        scalar2=total_b[:, r:r + 1],
                        op0=AT.add, op1=AT.min)
                    h0 = rwp.tile([128, T], f32, tag="h0")
                    nc.vector.tensor_scalar(
                        out=h0, in0=tgt, scalar1=cumT0[:, r:r + 1],
                        scalar2=0.0, op0=AT.subtract, op1=AT.max)
                    h1 = rwp.tile([128, T], f32, tag="h1")
                    nc.scalar.activation(
                        out=h1, in_=tgt, func=AF.Relu,
                        bias=ncumT1[:, r:r + 1], scale=1.0)
                    pslice = projbank[:, 3 * r:3 * r + 3]
                    nc.tensor.matmul(out=pslice, lhsT=h0,
                                     rhs=_view(dwT0, [[p, 3]], extra_off=r),
                                     start=True, stop=False)
                    nc.tensor.matmul(out=pslice, lhsT=h1,
                                     rhs=_view(dwT1, [[p, 3]], extra_off=r),
                                     start=False, stop=True)

                # --- epilogue: df = proj - posb (t-major), cost via matmul ---
                df = wp.tile([T, 3 * p], f32, tag="df")
                nc.vector.tensor_tensor(
                    out=_view(df, [[3, p], [1, 3]]),
                    in0=_view(projbank, [[3, p], [1, 3]]),
                    in1=_view(posbT, [[1, p], [p, 3]]),
                    op=AT.subtract)
                dfs.append(df)
                df2 = wp.tile([T, 3 * p], f32, tag="df2")
                nc.gpsimd.tensor_mul(out=df2, in0=df, in1=df)
                dd = wp.tile([T, p], f32, tag="dd")
                nc.vector.tensor_reduce(out=dd,
                                        in_=_view(df2, [[3, p], [1, 3]]),
                                        axis=AX.X, op=AT.add)
                dist = wp.tile([T, p], f32, tag="dist")
                nc.scalar.sqrt(out=dist, in_=dd)
                cps = pp.tile([1, p], f32, tag="tmp")
                nc.tensor.matmul(out=cps, lhsT=ones_col, rhs=dist,
                                 start=True, stop=True)
                nc.vector.tensor_copy(out=costS[:, r0:r0 + p], in_=cps)

            # --- per-sample argmin over branches ---
            nc.sync.dma_start(out=_dview(cost_d, [[1, rows]]), in_=costS)
            cost8 = fp.tile([ns, NB2], f32, tag="cost8")
            nc.sync.dma_start(out=cost8,
                              in_=_dview(cost_d, [[NB2, ns], [1, NB2]]))
            cmin = fp.tile([ns, 1], f32, tag="cmin")
            nc.vector.tensor_reduce(out=cmin, in_=cost8, axis=AX.X, op=AT.min)
            oh8 = fp.tile([ns, NB2], f32, tag="oh8")
            nc.vector.tensor_scalar(out=oh8, in0=cost8, scalar1=cmin,
                                    scalar2=None, op0=AT.is_equal)
            zc8 = fp.tile([ns, 1], f32, tag="zc8")
            nc.vector.memset(zc8, 0.0)
            pm8 = fp.tile([ns, NB2], f32, tag="pm8")
            nc.vector.tensor_tensor_scan(
                out=pm8, data0=oh8, data1=_view(zc8, [[0, NB2]]),
                initial=0.0, op0=AT.max, op1=AT.add)
            nc.vector.tensor_copy(out=oh8[:, 0:1], in_=pm8[:, 0:1])
            nc.vector.tensor_sub(out=oh8[:, 1:NB2], in0=pm8[:, 1:NB2],
                                 in1=pm8[:, 0:NB2 - 1])
            # roundtrip to a single-partition layout (matmul rhs needs base 0)
            oh_d = dp.tile([rows], f32)
            nc.sync.dma_start(out=_dview(oh_d, [[NB2, ns], [1, NB2]]),
                              in_=oh8)
            ohS = fp.tile([1, rows], f32, tag="ohS")
            nc.sync.dma_start(out=ohS, in_=_dview(oh_d, [[1, rows]]))

            # --- gather best branch per sample: out = df[best] + pos ---
            for n in range(ns):
                k = n // SPT
                rl = (n % SPT) * NB2   # local row offset within tile
                ohb = pp.tile([128, NB2], f32, tag="tmp")
                nc.tensor.matmul(out=ohb, lhsT=ones_row,
                                 rhs=ohS[:, NB2 * n:NB2 * (n + 1)],
                                 start=True, stop=True)
                pm = fp.tile([T, 3, NB2], f32, tag=f"pm{n % 2}")
                nc.vector.tensor_tensor(
                    out=pm,
                    in0=_view(dfs[k], [[1, 3], [3, NB2]], extra_off=3 * rl),
                    in1=_view(ohb, [[0, 3], [1, NB2]]),
                    op=AT.mult)
                outn = fp.tile([T, 3], f32, tag=f"outn{n % 2}")
                nc.vector.tensor_reduce(out=outn,
                                        in_=_view(pm, [[NB2, 3], [1, NB2]]),
                                        axis=AX.X, op=AT.add)
                outf = fp.tile([T, 3], f32, tag=f"outf{n % 2}")
                nc.vector.tensor_tensor(
                    out=outf, in0=outn,
                    in1=_view(tposTs[k], [[RT, 3]], extra_off=rl),
                    op=AT.add)
                nc.sync.dma_start(out=out_d.ap()[n], in_=outf)

    nc.compile()
    return nc


def marshal_inputs(selected_traj, road_points, road_mask):
    """Host-side layout marshaling (permutations/casts only): per-core input
    dicts with fwd+bwd branch rows and planar (xyz-major) layouts."""
    st = np.ascontiguousarray(selected_traj, dtype=np.float32)
    rp = np.ascontiguousarray(road_points, dtype=np.float32)
    rm = np.asarray(road_mask)

    rp_ext = np.concatenate([rp, rp[:, :, ::-1, :]], axis=1)        # [N,NB2,NP,3]
    rp_ext = np.ascontiguousarray(rp_ext.transpose(0, 1, 3, 2))     # [N,NB2,3,NP]
    mk_ext = np.concatenate([rm, rm[:, :, ::-1]], axis=1).astype(np.float32)
    tj = np.ascontiguousarray(st.transpose(0, 2, 1))                # [N,3,T]

    in_maps = []
    for c in range(NCORES):
        s = slice(c * NS, (c + 1) * NS)
        in_maps.append({
            "rp": np.ascontiguousarray(rp_ext[s]).reshape(NS * NB2, 3, NP),
            "mk": np.ascontiguousarray(mk_ext[s]).reshape(NS * NB2, NP),
            "tj": np.ascontiguousarray(tj[s]),
        })
    return in_maps


_NC = None


def kernel(selected_traj, road_points, road_mask):
    global _NC
    if _NC is None:
        _NC = build_nc()
    in_maps = marshal_inputs(selected_traj, road_points, road_mask)
    res = run_bass_kernel_spmd(_NC, in_maps, core_ids=list(range(NCORES)))
    out = np.concatenate([r["out"] for r in res.results], axis=0)
    return out.astype(np.float32)
